# revision 57
# baseline (speedup 1.0000x reference)
"""Trainium2 Bass kernel for nn_Encoder (tri-modal Mamba encoder), v2.

kernel(**inputs) takes FULL unsharded numpy inputs and returns the FULL
output (B, W, 2N+E, D). Batch B=8 is sharded across 8 NeuronCores (pure
data parallel, no collectives); params are replicated.

v2.1 vs v2: FFN LeakyReLU runs as Prelu (parametric_relu is resident in
every activation-table set, leaky_relu only in one), and the output path
transposes 512-col tiles (4 PE transposes -> one PSUM tile -> one copy ->
two DMAs) instead of per-2-seq chunks.

v2 vs v1:
- Phase A split per block into A1 (in_proj/conv/silu; Silu act table) and
  A2 (scan; exp/ln act table) to kill activation-table thrash.
- A[d,s] folded into the PE expansion weights so the dA exp has no
  per-group scale and runs 1024 columns wide across 2 PSUM banks.
- dt*x expanded to (d_lo, s) layout by DMA (DRAM bounce, replicating
  access pattern) instead of PE; u-multiply becomes a bf16 2x DVE op.
- x_proj emits dt_in/B/C in one (40,CT) matmul; B/C replicated across
  partitions by broadcast DMAs.
- Scan split DVE/Pool per state group; all elementwise muls bf16 2x.
- bf16 inter-phase spills; zero-bias/unit-gain folds detected at pack
  time are compiled out.
"""

import functools

import ml_dtypes
import numpy as np
from contextlib import ExitStack

import concourse.bass as bass
import concourse.tile as tile
import concourse.bacc as bacc_mod
import concourse.hw_specs as hw_specs_mod
from concourse import bacc, mybir
from concourse.bass_utils import run_bass_kernel_spmd

# Prefer the combined exp+ln activation table so the Exp/Ln mix (softplus
# factory, scan decay) resolves to one table set instead of thrashing
# between exp_and_others and natural_log on every alternation.
_GAT_ORIG = hw_specs_mod.get_activation_tables.__wrapped__


@functools.cache
def _gat_reordered(arch):
    t = dict(_GAT_ORIG(arch))
    pref = [k for k in ("natural_log_exp_and_others",) if k in t]
    return {k: t[k] for k in pref + [k for k in t if k not in pref]}


_ENABLE_GAT_REORDER = False
if _ENABLE_GAT_REORDER:
    hw_specs_mod.get_activation_tables = _gat_reordered
    bacc_mod.get_activation_tables = _gat_reordered

D, DI, SS, KK, RR = 128, 256, 16, 4, 8
B, W, N, E = 8, 64, 128, 256
Q = 128                      # seqs per block
CBLK = Q * W                 # 8192 cols per block
CT = 512                     # column tile (8 seqs)
NW = CBLK // 128
f32 = mybir.dt.float32
f32r = mybir.dt.float32r
bf16 = mybir.dt.bfloat16
AF = mybir.ActivationFunctionType
OP = mybir.AluOpType

BLOCKS = [("n", 0, "x_n", 0, 0),
          ("t0", 1, "x_t", 0, N),
          ("t1", 1, "x_t", Q, N + Q),
          ("l", 2, "x_l", 0, N + E)]
N_CORES = 8
LN_EPS = 1e-5
POOL_SCAN = ()   # scan is DVE-only on real HW


class Pack:
    def __init__(self):
        self.cols = []
        self.off = {}
        self.n = 0

    def add(self, name, arr, dtype=np.float32):
        arr = np.asarray(arr, dtype)
        assert arr.ndim == 2 and arr.shape[0] <= 128
        a = np.zeros((128, arr.shape[1]), dtype)
        a[: arr.shape[0]] = arr
        self.off[name] = (self.n, arr.shape[1])
        self.cols.append(a)
        self.n += arr.shape[1]

    def build(self):
        return np.concatenate(self.cols, axis=1)


def _host_pack(inp):
    """Returns (wp f32-staged-to-f32r, vp f32, bp bf16, flags)."""
    flags = {}
    dtb = np.asarray(inp["mp_dt_b"], np.float64)
    flags["dtb_const"] = float(dtb.flat[0]) if np.ptp(dtb) < 1e-12 else None
    flags["D_ones"] = bool(np.allclose(np.asarray(inp["mp_D"]), 1.0))
    flags["convb_zero"] = bool(np.all(np.asarray(inp["mp_conv_b"]) == 0.0))
    flags["f1b_zero"] = bool(np.all(np.asarray(inp["ff1_b"]) == 0.0))
    flags["f2b_zero"] = bool(np.all(np.asarray(inp["ff2_b"]) == 0.0))
    flags["an_id"] = bool(np.all(np.asarray(inp["an_g"]) == 1.0)
                          and np.all(np.asarray(inp["an_b"]) == 0.0))
    flags["fln_id"] = bool(np.all(np.asarray(inp["fln_g"]) == 1.0)
                           and np.all(np.asarray(inp["fln_b"]) == 0.0))
    flags["mixb_zero"] = bool(np.all(np.asarray(inp["mix_b"]) == 0.0))
    A = -np.exp(np.asarray(inp["mp_Alog"], np.float64))      # (3, DI, S)
    flags["A_shared"] = bool(
        np.ptp(A, axis=(0, 1)).max() < 1e-9 * np.abs(A).max())

    bp = Pack()   # bf16 weights
    for g in range(16):
        sm = np.zeros((128, 128), np.float32)
        for k in range(128):
            sm[k, g * 8 + k // 16] = 1.0
        bp.add(f"sum{g}", sm)

    def delta_A(Am, cc, g):
        # out partition j <- A[cc*128 + g*8 + j//16, j%16] placed at input
        # row g*8 + j//16 (row of the per-cc dts tile).
        dl = np.zeros((128, 128), np.float32)
        for j in range(128):
            dl[g * 8 + j // 16, j] = Am[cc * 128 + g * 8 + j // 16, j % 16]
        return dl

    if flags["A_shared"]:
        for g in range(16):
            bp.add(f"dA{g}", delta_A(A[0], 0, g))
    else:
        for m in range(3):
            for cc in range(2):
                for g in range(16):
                    bp.add(f"dA{m}{cc}{g}", delta_A(A[m], cc, g))

    wp = Pack()   # fp32 staged -> f32r on device
    vp = Pack()   # fp32 per-partition vectors
    for m in range(3):
        bp.add(f"win{m}", inp["mp_in"][m])                   # (D, 512)
        wxp = inp["mp_xproj"][m]                             # (DI, 40)
        for cc in range(2):
            bp.add(f"bc{m}{cc}", wxp[cc * 128:(cc + 1) * 128])   # (128, 40)
        dtw = inp["mp_dt_w"][m]                              # (R, DI)
        for cc in range(2):
            bp.add(f"dtw{m}{cc}", dtw[:, cc * 128:(cc + 1) * 128])
        wout = inp["mp_out"][m]                              # (DI, D)
        for cc in range(2):
            bp.add(f"wout{m}{cc}", wout[cc * 128:(cc + 1) * 128])
        bp.add(f"ff1{m}", inp["ff1_w"][m])                   # (D, 512)
        ff2 = inp["ff2_w"][m]                                # (4D, D)
        for c4 in range(4):
            bp.add(f"ff2{m}{c4}", ff2[c4 * 128:(c4 + 1) * 128])
    mixw = inp["mix_w"]
    for kc in range(2):
        for mc in range(2):
            bp.add(f"mix{kc}{mc}", mixw[kc * 128:(kc + 1) * 128,
                                        mc * 128:(mc + 1) * 128])
    wp.add("onesD", np.full((128, 1), 1.0 / D, np.float32))
    bp.add("onesDb", np.full((128, 1), 1.0 / D, np.float32))
    wp.add("ones1", np.ones((1, 128), np.float32))

    vp.add("eps", np.full((128, 1), LN_EPS, np.float32))
    if flags["dtb_const"] is not None:
        vp.add("dtbc", np.full((128, 1), flags["dtb_const"], np.float32))
    vp.add("I64", np.eye(64, dtype=np.float32))
    vp.add("I128", np.eye(128, dtype=np.float32))
    for m in range(3):
        cw = inp["mp_conv_w"][m]
        for cc in range(2):
            sl = slice(cc * 128, (cc + 1) * 128)
            vp.add(f"cw{m}{cc}", cw[sl])                     # 4 cols
            if not flags["convb_zero"]:
                vp.add(f"cb{m}{cc}", inp["mp_conv_b"][m][sl, None])
            if flags["dtb_const"] is None:
                vp.add(f"dtb{m}{cc}", inp["mp_dt_b"][m][sl, None])
            if not flags["D_ones"]:
                vp.add(f"Dp{m}{cc}", inp["mp_D"][m][sl, None])
        if not flags["f1b_zero"]:
            for c4 in range(4):
                vp.add(f"f1b{m}{c4}",
                       inp["ff1_b"][m][c4 * 128:(c4 + 1) * 128, None])
        if not flags["f2b_zero"]:
            vp.add(f"f2b{m}", inp["ff2_b"][m][:, None])
        if not flags["an_id"]:
            vp.add(f"ang{m}", inp["an_g"][m][:, None])
            vp.add(f"anb{m}", inp["an_b"][m][:, None])
        if not flags["fln_id"]:
            vp.add(f"flg{m}", inp["fln_g"][m][:, None])
            vp.add(f"flb{m}", inp["fln_b"][m][:, None])
    if not flags["mixb_zero"]:
        for mc in range(2):
            vp.add(f"mixb{mc}", inp["mix_b"][mc * 128:(mc + 1) * 128, None])
    return wp, vp, bp, flags


def _emit(ctx, tc, nc, aps, wp, vp, bpk, flags):
    wpool = ctx.enter_context(tc.tile_pool(name="weights", bufs=1))
    wr = wpool.tile([128, wp.n], f32r, name="wr", tag="wr")
    vec = wpool.tile([128, vp.n], f32, name="vec", tag="vec")
    nc.sync.dma_start(vec[:], aps["vpack"][:])
    bw = wpool.tile([128, bpk.n], bf16, name="bw", tag="bw")
    nc.sync.dma_start(bw[:], aps["bpack"][:])
    with tc.tile_pool(name="wstage", bufs=1) as stpool:
        wstage = stpool.tile([128, wp.n], f32, name="wstage")
        nc.sync.dma_start(wstage[:], aps["wpack"][:])
        for o in range(0, wp.n, 8192):
            e = min(wp.n, o + 8192)
            nc.vector.tensor_copy(wr[:, o:e], wstage[:, o:e])

    def WR(name):
        o, c = wp.off[name]
        return wr[:, o:o + c]

    def VP(name):
        o, c = vp.off[name]
        return vec[:, o:o + c]

    def BR(name):
        o, c = bpk.off[name]
        return bw[:, o:o + c]

    def mm(psum_ap, lhsT_ap, rhs_ap, start, stop, kp=128):
        nc.tensor.matmul(psum_ap, lhsT_ap[:kp, :], rhs_ap[:kp, :],
                         start=start, stop=stop)

    def dAW(mi, cc, g):
        return BR(f"dA{g}" if flags["A_shared"] else f"dA{mi}{cc}{g}")

    I64 = VP("I64")
    I128 = VP("I128")

    statA = {bname: aps[f"scr_stA_{bname}"] for bname, _, _, _, _ in BLOCKS}

    stpool = ctx.enter_context(tc.tile_pool(name="stat_s", bufs=2))

    def ln_stats(src_ap, sq_ap, statT, c0, ppool, ptag, pbufs=2):
        """src (128,CT); sq squared values; write stats into statT."""
        ob = BR("onesDb")[:, 0:1]
        of = WR("onesD")[:, 0:1]
        pmq = ppool.tile([1, 2 * CT], f32, name="pmq", tag=ptag, bufs=pbufs)
        mm(pmq[:, 0:CT], ob if src_ap.dtype == bf16 else of, src_ap,
           True, True)
        mm(pmq[:, CT:2 * CT], ob if sq_ap.dtype == bf16 else of, sq_ap,
           True, True)
        sst = stpool.tile([1, 2 * CT], f32, name="sst", tag="sst")
        nc.scalar.activation(sst[:], pmq[:], AF.Copy)
        p0 = c0 // NW
        nc.sync.dma_start(
            statT[p0:p0 + 8].rearrange("p h w -> h p w"),
            sst[:].rearrange("x (h p w) -> x h p w", h=2, p=8))

    # ================= phase A =====================================
    def phase_a(bname, mi, xkey, q_off, sqp):
        is_t = bname in ("t0", "t1")
        xcb = [sqp.tile([128, CBLK], bf16, name=f"xcb{cc}", tag=f"xcb{cc}")
               for cc in range(2)]
        zsb = [sqp.tile([128, CBLK], bf16, name=f"zsb{cc}", tag=f"zsb{cc}")
               for cc in range(2)]
        xT = sqp.tile([128, CBLK], bf16, name="xT", tag="xT")

        # ---- A1 (with fused per-tile input transpose) ------------
        with tc.tile_pool(name="a0", bufs=3) as a0p, \
             tc.tile_pool(name="a0p", bufs=2, space="PSUM") as a0pp, \
             tc.tile_pool(name="a1", bufs=2) as a1p, \
             tc.tile_pool(name="a1pxc", bufs=1, space="PSUM") as pxc, \
             tc.tile_pool(name="a1pz", bufs=1, space="PSUM") as pz_, \
             tc.tile_pool(name="a1pbc", bufs=2, space="PSUM") as pbcp:
            for c0 in range(0, CBLK, CT):
                q0 = c0 // W
                raw = a0p.tile([64, 8 * 128], f32, name="raw", tag="raw")
                nc.sync.dma_start(raw[:],
                                  aps[xkey][:, q_off + q0:q_off + q0 + 8, :])
                pt = a0pp.tile([128, 8 * 64], f32, name="pt", tag="pt")
                for i in range(8):
                    nc.tensor.transpose(pt[:, i * 64:(i + 1) * 64],
                                        raw[:, i * 128:(i + 1) * 128],
                                        I64[:64, :64])
                nc.scalar.activation(xT[:, c0:c0 + CT], pt[:], AF.Copy)
                xt_t = xT[:, c0:c0 + CT]
                pxc2 = pxc.tile([128, 2 * CT], f32, name="pxc2", tag="pxc2")
                for cc in range(2):
                    mm(pxc2[:, cc * CT:(cc + 1) * CT],
                       BR(f"win{mi}")[:, cc * 128:(cc + 1) * 128], xt_t,
                       True, True)
                pz2 = pz_.tile([128, 2 * CT], f32, name="pz2", tag="pz2")
                for cc in range(2):
                    mm(pz2[:, cc * CT:(cc + 1) * CT],
                       BR(f"win{mi}")[:, (2 + cc) * 128:(3 + cc) * 128],
                       xt_t, True, True)
                for cc in range(2):
                    nc.scalar.activation(zsb[cc][:, c0:c0 + CT],
                                         pz2[:, cc * CT:(cc + 1) * CT],
                                         AF.Silu)
                zc = a1p.tile([128, 2 * CT], bf16, name="zc", tag="zc")
                nc.scalar.activation(zc[:], pxc2[:], AF.Copy)
                acc2 = a1p.tile([128, 2 * CT], f32, name="acc2", tag="acc2")
                for cc in range(2):
                    pzv = zc[:, cc * CT:(cc + 1) * CT]
                    accv = acc2[:, cc * CT:(cc + 1) * CT]
                    cw = VP(f"cw{mi}{cc}")
                    srcr = pzv.rearrange("p (q t) -> p q t", t=W)
                    accr = accv.rearrange("p (q t) -> p q t", t=W)
                    nc.vector.tensor_scalar(accv, pzv, cw[:, 3:4], None,
                                            OP.mult)
                    for k in range(3):
                        sh = 3 - k
                        eng = nc.vector
                        eng.scalar_tensor_tensor(
                            accr[:, :, sh:W], srcr[:, :, 0:W - sh],
                            cw[:, k:k + 1], accr[:, :, sh:W],
                            OP.mult, OP.add)
                for cc in range(2):
                    bias = (None if flags["convb_zero"]
                            else VP(f"cb{mi}{cc}"))
                    if bias is None:
                        nc.scalar.activation(xcb[cc][:, c0:c0 + CT],
                                             acc2[:, cc * CT:(cc + 1) * CT],
                                             AF.Silu)
                    else:
                        nc.scalar.activation(xcb[cc][:, c0:c0 + CT],
                                             acc2[:, cc * CT:(cc + 1) * CT],
                                             AF.Silu, bias=bias)
                pbc = pbcp.tile([40, CT], f32, name="pbc", tag="pbc")
                for cc in range(2):
                    mm(pbc[:], BR(f"bc{mi}{cc}"),
                       xcb[cc][:, c0:c0 + CT], cc == 0, cc == 1)
                bcs = a1p.tile([40, CT], bf16, name="bcs", tag="bcs")
                nc.scalar.activation(bcs[:], pbc[:], AF.Copy)
                nc.sync.dma_start(aps[f"scr_bc_{bname}"][:, c0:c0 + CT],
                                  bcs[:])

        # ---- A2 loop 1: dt factory (Exp/Ln batched by 4) ---------
        with tc.tile_pool(name="f1", bufs=2) as f1p, \
             tc.tile_pool(name="f1ps", bufs=2, space="PSUM") as f1ps:
            for c00 in range(0, CBLK, 4 * CT):
                ezs = []
                for ci in range(4):
                    c0 = c00 + ci * CT
                    dtin = f1p.tile([8, CT], bf16, name="dtin", tag="dtin",
                                    bufs=4)
                    nc.sync.dma_start(dtin[:],
                                      aps[f"scr_bc_{bname}"][0:8, c0:c0 + CT])
                    pd = f1ps.tile([128, 2 * CT], f32, name="pd", tag="pd",
                                   bufs=2)
                    for cc in range(2):
                        mm(pd[:, cc * CT:(cc + 1) * CT], BR(f"dtw{mi}{cc}"),
                           dtin[:], True, True, kp=8)
                    ez = f1p.tile([128, 2 * CT], bf16, name="ez", tag="ez",
                                  bufs=4)
                    if flags["dtb_const"] is not None:
                        nc.scalar.activation(ez[:], pd[:], AF.Exp,
                                             bias=VP("dtbc"))
                    else:
                        for cc in range(2):
                            nc.scalar.activation(
                                ez[:, cc * CT:(cc + 1) * CT],
                                pd[:, cc * CT:(cc + 1) * CT],
                                AF.Exp, bias=VP(f"dtb{mi}{cc}"))
                    ezs.append(ez)
                for ci in range(4):
                    c0 = c00 + ci * CT
                    dts = f1p.tile([128, 2 * CT], bf16, name="dts",
                                   tag="dts", bufs=4)
                    nc.scalar.activation(dts[:], ezs[ci][:], AF.Ln, bias=1.0)
                    dtx = f1p.tile([128, 2 * CT], bf16, name="dtx",
                                   tag="dtx", bufs=4)
                    for cc in range(2):
                        nc.gpsimd.tensor_mul(dtx[:, cc * CT:(cc + 1) * CT],
                                             dts[:, cc * CT:(cc + 1) * CT],
                                             xcb[cc][:, c0:c0 + CT])
                    for cc in range(2):
                        nc.sync.dma_start(
                            aps[f"scr_dtx_{bname}"][cc, c0 // CT]
                            .rearrange("p (g c) -> g p c", g=16),
                            dtx[:, cc * CT:(cc + 1) * CT])
                    dtv = dts[:].rearrange("p (x t) -> p x t", t=W)
                    nc.vector.tensor_scalar(dtv[:, :, 0:1], dtv[:, :, 0:1],
                                            0.0, 1.0e4, OP.mult, OP.add)
                    nc.sync.dma_start(aps[f"scr_dts_{bname}"][:, :, c0:c0 + CT],
                                      dts[:].rearrange("p (cc c) -> p cc c",
                                                       cc=2))

        # ---- A2 loop 2: selective scan (exp table only) ----------
        with tc.tile_pool(name="a2", bufs=2) as a2p, \
             tc.tile_pool(name="a2u", bufs=4) as a2u, \
             tc.tile_pool(name="a2x", bufs=6) as a2x, \
             tc.tile_pool(name="a2pda", bufs=2, space="PSUM") as pda, \
             tc.tile_pool(name="a2ppy", bufs=2, space="PSUM") as ppy, \
             tc.tile_pool(name="a2pmisc", bufs=1, space="PSUM") as pmsc:
            for c0 in range(0, CBLK, CT):
                dtl = a2p.tile([128, 2 * CT], bf16, name="dtl", tag="dtl",
                               bufs=4)
                nc.sync.dma_start(
                    dtl[:].rearrange("p (cc c) -> p cc c", cc=2),
                    aps[f"scr_dts_{bname}"][:, :, c0:c0 + CT])
                Brep = a2p.tile([128, CT], bf16, name="Brep", tag="Brep",
                                bufs=4)
                nc.sync.dma_start(
                    Brep[:],
                    aps[f"scr_bc_{bname}"][8:24, c0:c0 + CT]
                    .unsqueeze(0).broadcast_to((8, 16, CT)))
                Crep = a2p.tile([128, CT], bf16, name="Crep", tag="Crep",
                                bufs=4)
                nc.sync.dma_start(
                    Crep[:],
                    aps[f"scr_bc_{bname}"][24:40, c0:c0 + CT]
                    .unsqueeze(0).broadcast_to((8, 16, CT)))
                ues = []
                for cc in range(2):
                    halves = []
                    for hf in range(2):
                        ueh = a2x.tile([128, 8 * CT], bf16, name="ueh",
                                       tag="ueh", bufs=4)
                        nc.sync.dma_start(
                            ueh[:],
                            aps[f"scr_dtx_{bname}"]
                            [cc, c0 // CT, :, hf * 8 * CT:(hf + 1) * 8 * CT]
                            .unsqueeze(1).broadcast_to((8, 16, 8 * CT)))
                        halves.append(ueh)
                    ues.append(halves)
                Brep_b = Brep[:].unsqueeze(1).broadcast_to((128, 2, CT))
                Crep_b = Crep[:].unsqueeze(1).broadcast_to((128, 2, CT))
                gzs = []
                for cc in range(2):
                    pY = ppy.tile([128, CT], f32, name="pY", tag="pY",
                                  bufs=2)
                    if flags["D_ones"]:
                        # pre-load the D-skip term; the sum matmuls
                        # accumulate on top (start=False), removing the
                        # DVE add from the scan window.
                        nc.scalar.activation(pY[:],
                                             xcb[cc][:, c0:c0 + CT],
                                             AF.Copy)
                    stage = []
                    for gp in range(11):
                        if gp < 8:
                            pP = pda.tile([128, 2 * CT], f32, name="pP",
                                          tag="pP", bufs=2)
                            for i in range(2):
                                g = gp * 2 + i
                                mm(pP[:, i * CT:(i + 1) * CT],
                                   dAW(mi, cc, g),
                                   dtl[:, cc * CT:(cc + 1) * CT],
                                   True, True)
                            dA = a2u.tile([128, 2 * CT], bf16, name="dA",
                                          tag="dA", bufs=4)
                            nc.scalar.activation(dA[:], pP[:], AF.Exp)
                            u2 = a2u.tile([128, 2 * CT], bf16, name="u2",
                                          tag="u2", bufs=5)
                            ueh = ues[cc][gp // 4]
                            sl = (gp % 4) * 2 * CT
                            if gp % 4 == 3:
                                for i in range(2):
                                    nc.gpsimd.tensor_mul(
                                        u2[:, i * CT:(i + 1) * CT],
                                        ueh[:, sl + i * CT:sl + (i + 1) * CT],
                                        Brep[:])
                            else:
                                nc.vector.tensor_mul(
                                    u2[:].rearrange("p (i c) -> p i c", i=2),
                                    ueh[:, sl:sl + 2 * CT]
                                    .rearrange("p (i c) -> p i c", i=2),
                                    Brep_b)
                            stage.append((gp, dA, u2))
                        if gp >= 3:
                            gq, dAq, u2q = stage.pop(0)
                            h2 = a2u.tile([128, 2 * CT], bf16, name="h2",
                                          tag="h2", bufs=4)
                            for i in range(2):
                                nc.vector.tensor_tensor_scan(
                                    h2[:, i * CT:(i + 1) * CT],
                                    dAq[:, i * CT:(i + 1) * CT],
                                    u2q[:, i * CT:(i + 1) * CT],
                                    0.0, OP.mult, OP.add)
                            yh2 = a2u.tile([128, 2 * CT], bf16, name="yh2",
                                           tag="yh2", bufs=4)
                            if gq % 2 == 1:
                                for i in range(2):
                                    nc.gpsimd.tensor_mul(
                                        yh2[:, i * CT:(i + 1) * CT],
                                        h2[:, i * CT:(i + 1) * CT],
                                        Crep[:])
                            else:
                                nc.vector.tensor_mul(
                                    yh2[:].rearrange("p (i c) -> p i c",
                                                     i=2),
                                    h2[:].rearrange("p (i c) -> p i c",
                                                    i=2),
                                    Crep_b)
                            for i in range(2):
                                g = gq * 2 + i
                                mm(pY[:], BR(f"sum{g}"),
                                   yh2[:, i * CT:(i + 1) * CT],
                                   g == 0 and not flags["D_ones"],
                                   g == 15)
                    yg = a2p.tile([128, CT], bf16, name=f"yg{cc}",
                                  tag=f"yg{cc}")
                    if flags["D_ones"]:
                        # pY already holds sum+skip; drain on Act (slack)
                        nc.scalar.activation(yg[:], pY[:], AF.Copy)
                    else:
                        nc.vector.scalar_tensor_tensor(
                            yg[:], xcb[cc][:, c0:c0 + CT],
                            VP(f"Dp{mi}{cc}"), pY[:], OP.mult, OP.add)
                    gz = a2p.tile([128, CT], bf16, name=f"gz{cc}",
                                  tag=f"gz{cc}")
                    nc.gpsimd.tensor_mul(gz[:], yg[:],
                                         zsb[cc][:, c0:c0 + CT])
                    gzs.append(gz)
                po = pmsc.tile([128, CT], f32, name="po", tag="pdpo",
                               bufs=2)
                for cc in range(2):
                    mm(po[:], BR(f"wout{mi}{cc}"), gzs[cc][:],
                       cc == 0, cc == 1)
                res = a2p.tile([128, CT], bf16, name="res", tag="res")
                if is_t:
                    nc.vector.tensor_add(res[:], po[:], xT[:, c0:c0 + CT])
                    nc.sync.dma_start(aps[f"scr_s_{bname}"][:, c0:c0 + CT],
                                      res[:])
                    s2 = a2p.tile([128, CT], bf16, name="s2", tag="s2")
                    nc.scalar.activation(s2[:], res[:], AF.Square)
                    ln_stats(res[:], s2[:], statA[bname], c0, pda, "pP")
                else:
                    nc.scalar.activation(res[:], po[:], AF.Copy)
                    nc.sync.dma_start(aps[f"scr_ym_{bname}"][:, c0:c0 + CT],
                                      res[:])

    ABLOCKS = {b[0]: b for b in BLOCKS}
    with tc.tile_pool(name="a_sq", bufs=1) as a_sqp:
        for bname in ("n", "l"):
            bn, mi, xkey, q_off, _ = ABLOCKS[bname]
            phase_a(bn, mi, xkey, q_off, a_sqp)
        # ---- mix phase (n, l) -----------------------------------
        with tc.tile_pool(name="mixw", bufs=2) as mxp, \
             tc.tile_pool(name="mixp", bufs=1, space="PSUM") as mxpp:
            for c0 in range(0, CBLK, CT):
                q0 = c0 // W
                cat = []
                for bname in ("n", "l"):
                    t_ = mxp.tile([128, CT], bf16, name=f"ym{bname}",
                                  tag=f"ym{bname}")
                    nc.sync.dma_start(t_[:],
                                      aps[f"scr_ym_{bname}"][:, c0:c0 + CT])
                    cat.append(t_)
                for mc, bname in enumerate(("n", "l")):
                    pmx = mxpp.tile([128, CT], f32, name="pmx", tag="pmx",
                                    bufs=2)
                    for kc in range(2):
                        mm(pmx[:], BR(f"mix{kc}{mc}"), cat[kc][:], kc == 0,
                           kc == 1)
                    ms = mxp.tile([128, CT], bf16, name="ms", tag="ms")
                    if flags["mixb_zero"]:
                        nc.scalar.activation(ms[:], pmx[:], AF.Silu)
                    else:
                        nc.scalar.activation(ms[:], pmx[:], AF.Silu,
                                             bias=VP(f"mixb{mc}"))
                    raw = mxp.tile([64, 8 * 128], f32, name="rawm",
                                   tag="rawm")
                    nc.sync.dma_start(raw[:],
                                      aps[f"x_{bname}"][:, q0:q0 + 8, :])
                    ptx = mxpp.tile([128, CT], f32, name="ptx", tag="pmx",
                                    bufs=2)
                    for i in range(8):
                        nc.tensor.transpose(ptx[:, i * 64:(i + 1) * 64],
                                            raw[:, i * 128:(i + 1) * 128],
                                            I64[:64, :64])
                    t2 = mxp.tile([128, CT], bf16, name="t2", tag="t2")
                    nc.vector.tensor_add(t2[:], cat[mc][:], ms[:])
                    res = mxp.tile([128, CT], bf16, name="resm", tag="resm")
                    nc.vector.tensor_add(res[:], t2[:], ptx[:])
                    nc.sync.dma_start(aps[f"scr_s_{bname}"][:, c0:c0 + CT],
                                      res[:])
                    s2m = mxp.tile([128, CT], bf16, name="s2m", tag="s2m")
                    nc.scalar.activation(s2m[:], res[:], AF.Square)
                    ln_stats(res[:], s2m[:], statA[bname], c0, mxpp, "pmx")
        for bname in ("t0", "t1"):
            bn, mi, xkey, q_off, _ = ABLOCKS[bname]
            phase_a(bn, mi, xkey, q_off, a_sqp)

    # ================= phase C =====================================
    def ln_finish(tag, statT, pool, scr):
        stl = pool.tile([128, 2 * NW], f32, name=f"stl_{tag}", tag="stl")
        nc.sync.dma_start(stl[:].rearrange("p (h w) -> p h w", h=2),
                          statT[:])
        m_t, q_t = stl[:, 0:NW], stl[:, NW:2 * NW]
        var = pool.tile([128, NW], f32, name=f"var_{tag}", tag="lnvar")
        nc.vector.tensor_mul(var[:], m_t, m_t)
        nc.vector.tensor_sub(var[:], q_t, var[:])
        sd = pool.tile([128, NW], f32, name=f"sd_{tag}", tag="lnsd")
        nc.scalar.activation(sd[:], var[:], AF.Sqrt, bias=VP("eps"))
        r_t = pool.tile([128, NW], f32r, name=f"r_{tag}", tag="lnr")
        with nc.allow_low_precision(reason="f32r LN scale factors"):
            nc.vector.reciprocal(r_t[:], sd[:])
        nmr = pool.tile([128, NW], f32r, name=f"nmr_{tag}", tag="lnnmr")
        nc.vector.tensor_mul(nmr[:], m_t, r_t[:])
        nc.vector.tensor_scalar(nmr[:], nmr[:], -1.0, None, OP.mult)
        rnm = pool.tile([128, 2 * NW], bf16, name=f"rnm_{tag}", tag="lnrnm")
        nc.vector.tensor_copy(rnm[:, 0:NW], r_t[:])
        nc.vector.tensor_copy(rnm[:, NW:2 * NW], nmr[:])
        nc.sync.dma_start(scr.rearrange("h p w -> p h w"), rnm[:])
        return r_t, nmr

    def ln_apply(src_ap, scr, c0, gk, bk, out_ap, pool):
        p0 = c0 // NW
        rnm2 = pool.tile([128, 2 * CT], bf16, name="rnm2", tag="rnm2",
                         bufs=3)
        nc.sync.dma_start(
            rnm2[:].rearrange("j (h ab) -> j h ab", h=2),
            scr[:, p0:p0 + 8, :].rearrange("h p w -> h (p w)")
            .unsqueeze(0).broadcast_to((128, 2, CT)))
        t1 = pool.tile([128, CT], bf16, name="t1", tag="t1")
        nc.vector.tensor_mul(t1[:], src_ap, rnm2[:, 0:CT])
        if gk is None:
            nc.vector.tensor_add(out_ap, t1[:], rnm2[:, CT:2 * CT])
        else:
            nc.vector.tensor_add(t1[:], t1[:], rnm2[:, CT:2 * CT])
            nc.vector.tensor_scalar(out_ap, t1[:], VP(gk), VP(bk),
                                    OP.mult, OP.add)

    with tc.tile_pool(name="c_per", bufs=1) as cper, \
         tc.tile_pool(name="c_w", bufs=2) as cp, \
         tc.tile_pool(name="c_p", bufs=1, space="PSUM") as cpp, \
         tc.tile_pool(name="c_pf", bufs=1, space="PSUM") as cpf:
        state = {}

        def c_front(bname, mi):
            ln_finish(f"a{bname}", statA[bname], cp,
                      aps[f"scr_ln_{bname}_a"][:])
            statF = aps[f"scr_stF_{bname}"]
            n1 = cper.tile([128, CBLK], bf16, name=f"n1_{bname}",
                           tag="napply", bufs=2)
            sf = cper.tile([128, CBLK], bf16, name=f"sf_{bname}",
                           tag="sfb", bufs=3)
            for c0 in range(0, CBLK, CT):
                sld = cp.tile([128, CT], bf16, name="sld", tag="sld",
                              bufs=3)
                nc.sync.dma_start(sld[:],
                                  aps[f"scr_s_{bname}"][:, c0:c0 + CT])
                ga, gb = (None, None) if flags["an_id"] else \
                    (f"ang{mi}", f"anb{mi}")
                ln_apply(sld[:], aps[f"scr_ln_{bname}_a"], c0, ga, gb,
                         n1[:, c0:c0 + CT], cp)
                hh = cp.tile([128, 4 * CT], bf16, name="hh", tag="hh")
                for hp in range(2):
                    pf = cpf.tile([128, 2 * CT], f32, name="pf", tag="pf",
                                  bufs=2)
                    for ci in range(2):
                        c4 = hp * 2 + ci
                        mm(pf[:, ci * CT:(ci + 1) * CT],
                           BR(f"ff1{mi}")[:, c4 * 128:(c4 + 1) * 128],
                           n1[:, c0:c0 + CT], True, True)
                    if flags["f1b_zero"]:
                        nc.scalar.activation(
                            hh[:, hp * 2 * CT:(hp + 1) * 2 * CT], pf[:],
                            AF.Prelu, alpha=0.01)
                    else:
                        for ci in range(2):
                            c4 = hp * 2 + ci
                            nc.scalar.activation(
                                hh[:, c4 * CT:(c4 + 1) * CT],
                                pf[:, ci * CT:(ci + 1) * CT], AF.Prelu,
                                bias=VP(f"f1b{mi}{c4}"), alpha=0.01)
                pf2 = cpp.tile([128, CT], f32, name="pf2", tag="pf2",
                               bufs=2)
                for c4 in range(4):
                    mm(pf2[:], BR(f"ff2{mi}{c4}"),
                       hh[:, c4 * CT:(c4 + 1) * CT], c4 == 0, c4 == 3)
                if flags["f2b_zero"]:
                    nc.vector.tensor_add(sf[:, c0:c0 + CT], pf2[:],
                                         n1[:, c0:c0 + CT])
                else:
                    nc.vector.scalar_tensor_tensor(sf[:, c0:c0 + CT],
                                                   pf2[:], VP(f"f2b{mi}"),
                                                   n1[:, c0:c0 + CT],
                                                   OP.add, OP.add)
                s2f = cp.tile([128, CT], bf16, name="s2f", tag="s2f")
                nc.scalar.activation(s2f[:], sf[:, c0:c0 + CT],
                                     AF.Square)
                ln_stats(sf[:, c0:c0 + CT], s2f[:], statF, c0, cpf, "pf",
                         pbufs=2)
            state[bname] = (statF, sf)

        def c_tail(bname, mi, j0):
            statF, sf = state.pop(bname)
            ln_finish(f"f{bname}", statF, cp,
                      aps[f"scr_ln_{bname}_f"][:])
            n2 = cper.tile([128, CBLK], f32, name=f"n2_{bname}",
                           tag="napply", bufs=2)
            for c0 in range(0, CBLK, CT):
                ga, gb = (None, None) if flags["fln_id"] else \
                    (f"flg{mi}", f"flb{mi}")
                ln_apply(sf[:, c0:c0 + CT], aps[f"scr_ln_{bname}_f"], c0,
                         ga, gb, n2[:, c0:c0 + CT], cp)
            for q0 in range(0, Q, 8):
                c0 = q0 * 64
                pt = cpp.tile([128, 512], f32, name="pto", tag="pto",
                              bufs=2)
                for c in range(4):
                    nc.tensor.transpose(
                        pt[:, c * 128:(c + 1) * 128],
                        n2[:, c0 + c * 128:c0 + (c + 1) * 128], I128)
                ot = cp.tile([128, 512], f32, name="ot", tag="ot")
                nc.vector.tensor_copy(ot[:], pt[:])
                for qh in range(2):
                    nc.sync.dma_start(
                        aps["out"][:, j0 + q0:j0 + q0 + 8, :]
                        .rearrange("t (c q) d -> q t c d", c=4)[qh],
                        ot[qh * 64:(qh + 1) * 64, :]
                        .rearrange("t (c d) -> t c d", c=4))

        binfo = {b[0]: b for b in BLOCKS}
        order = ["n", "t0", "t1", "l"]
        prev = None
        for bname in order:
            _, mi, _, _, j0 = binfo[bname]
            c_front(bname, mi)
            if prev is not None:
                _, pmi, _, _, pj0 = binfo[prev]
                c_tail(prev, pmi, pj0)
            prev = bname
        _, pmi, _, _, pj0 = binfo[prev]
        c_tail(prev, pmi, pj0)


def _build_program(wp, vp, bpk, flags):
    nc = bacc.Bacc("TRN2", target_bir_lowering=False, debug=False,
                   num_devices=N_CORES)
    aps = {}
    aps["x_n"] = nc.dram_tensor("x_n", [W, N, D], f32,
                                kind="ExternalInput").ap()
    aps["x_t"] = nc.dram_tensor("x_t", [W, E, D], f32,
                                kind="ExternalInput").ap()
    aps["x_l"] = nc.dram_tensor("x_l", [W, N, D], f32,
                                kind="ExternalInput").ap()
    aps["wpack"] = nc.dram_tensor("wpack", [128, wp.n], f32,
                                  kind="ExternalInput").ap()
    aps["vpack"] = nc.dram_tensor("vpack", [128, vp.n], f32,
                                  kind="ExternalInput").ap()
    aps["bpack"] = nc.dram_tensor("bpack", [128, bpk.n], bf16,
                                  kind="ExternalInput").ap()
    aps["out"] = nc.dram_tensor("out", [W, 2 * N + E, D], f32,
                                kind="ExternalOutput").ap()
    for bname, _, _, _, _ in BLOCKS:
        aps[f"scr_bc_{bname}"] = nc.dram_tensor(
            f"scr_bc_{bname}", [40, CBLK], bf16).ap()
        aps[f"scr_dtx_{bname}"] = nc.dram_tensor(
            f"scr_dtx_{bname}", [2, CBLK // CT, 8, 16 * CT], bf16).ap()
        aps[f"scr_stA_{bname}"] = nc.dram_tensor(
            f"scr_stA_{bname}", [128, 2, NW], f32).ap()
        aps[f"scr_stF_{bname}"] = nc.dram_tensor(
            f"scr_stF_{bname}", [128, 2, NW], f32).ap()
        aps[f"scr_ln_{bname}_a"] = nc.dram_tensor(
            f"scr_ln_{bname}_a", [2, 128, NW], bf16).ap()
        aps[f"scr_ln_{bname}_f"] = nc.dram_tensor(
            f"scr_ln_{bname}_f", [2, 128, NW], bf16).ap()
        aps[f"scr_dts_{bname}"] = nc.dram_tensor(
            f"scr_dts_{bname}", [128, 2, CBLK], bf16).ap()
        aps[f"scr_s_{bname}"] = nc.dram_tensor(
            f"scr_s_{bname}", [128, CBLK], bf16).ap()
        if bname in ("n", "l"):
            aps[f"scr_ym_{bname}"] = nc.dram_tensor(
                f"scr_ym_{bname}", [128, CBLK], bf16).ap()

    with tile.TileContext(nc) as tc:
        with ExitStack() as ctx:
            _emit(ctx, tc, nc, aps, wp, vp, bpk, flags)
    nc.compile()
    return nc


_CACHE = {}


def kernel(**inputs):
    wp, vp, bpk, flags = _host_pack(inputs)
    if "prog" not in _CACHE:
        _CACHE["prog"] = _build_program(wp, vp, bpk, flags)
    nc = _CACHE["prog"]
    wpack, vpack = wp.build(), vp.build()
    bpack = bpk.build().astype(ml_dtypes.bfloat16)
    in_maps = []
    for b in range(B):
        in_maps.append({
            "x_n": np.ascontiguousarray(inputs["x_node"][b]),
            "x_t": np.ascontiguousarray(inputs["x_trace"][b]),
            "x_l": np.ascontiguousarray(inputs["x_log"][b]),
            "wpack": wpack,
            "vpack": vpack,
            "bpack": bpack,
        })
    res = run_bass_kernel_spmd(nc, in_maps, list(range(N_CORES)))
    out = np.stack([res.results[b]["out"] for b in range(B)], axis=0)
    return out.astype(np.float32)



# revision 59
# speedup vs baseline: 1.0049x; 1.0049x over previous
"""Trainium2 Bass kernel for nn_Encoder (tri-modal Mamba encoder), v2.

kernel(**inputs) takes FULL unsharded numpy inputs and returns the FULL
output (B, W, 2N+E, D). Batch B=8 is sharded across 8 NeuronCores (pure
data parallel, no collectives); params are replicated.

v2.1 vs v2: FFN LeakyReLU runs as Prelu (parametric_relu is resident in
every activation-table set, leaky_relu only in one), and the output path
transposes 512-col tiles (4 PE transposes -> one PSUM tile -> one copy ->
two DMAs) instead of per-2-seq chunks.

v2 vs v1:
- Phase A split per block into A1 (in_proj/conv/silu; Silu act table) and
  A2 (scan; exp/ln act table) to kill activation-table thrash.
- A[d,s] folded into the PE expansion weights so the dA exp has no
  per-group scale and runs 1024 columns wide across 2 PSUM banks.
- dt*x expanded to (d_lo, s) layout by DMA (DRAM bounce, replicating
  access pattern) instead of PE; u-multiply becomes a bf16 2x DVE op.
- x_proj emits dt_in/B/C in one (40,CT) matmul; B/C replicated across
  partitions by broadcast DMAs.
- Scan split DVE/Pool per state group; all elementwise muls bf16 2x.
- bf16 inter-phase spills; zero-bias/unit-gain folds detected at pack
  time are compiled out.
"""

import functools

import ml_dtypes
import numpy as np
from contextlib import ExitStack

import concourse.bass as bass
import concourse.tile as tile
import concourse.bacc as bacc_mod
import concourse.hw_specs as hw_specs_mod
from concourse import bacc, mybir
from concourse.bass_utils import run_bass_kernel_spmd

# Prefer the combined exp+ln activation table so the Exp/Ln mix (softplus
# factory, scan decay) resolves to one table set instead of thrashing
# between exp_and_others and natural_log on every alternation.
_GAT_ORIG = hw_specs_mod.get_activation_tables.__wrapped__


@functools.cache
def _gat_reordered(arch):
    t = dict(_GAT_ORIG(arch))
    pref = [k for k in ("natural_log_exp_and_others",) if k in t]
    return {k: t[k] for k in pref + [k for k in t if k not in pref]}


_ENABLE_GAT_REORDER = False
if _ENABLE_GAT_REORDER:
    hw_specs_mod.get_activation_tables = _gat_reordered
    bacc_mod.get_activation_tables = _gat_reordered

D, DI, SS, KK, RR = 128, 256, 16, 4, 8
B, W, N, E = 8, 64, 128, 256
Q = 128                      # seqs per block
CBLK = Q * W                 # 8192 cols per block
CT = 512                     # column tile (8 seqs)
NW = CBLK // 128
f32 = mybir.dt.float32
f32r = mybir.dt.float32r
bf16 = mybir.dt.bfloat16
AF = mybir.ActivationFunctionType
OP = mybir.AluOpType

BLOCKS = [("n", 0, "x_n", 0, 0),
          ("t0", 1, "x_t", 0, N),
          ("t1", 1, "x_t", Q, N + Q),
          ("l", 2, "x_l", 0, N + E)]
N_CORES = 8
LN_EPS = 1e-5
POOL_SCAN = ()   # scan is DVE-only on real HW


class Pack:
    def __init__(self):
        self.cols = []
        self.off = {}
        self.n = 0

    def add(self, name, arr, dtype=np.float32):
        arr = np.asarray(arr, dtype)
        assert arr.ndim == 2 and arr.shape[0] <= 128
        a = np.zeros((128, arr.shape[1]), dtype)
        a[: arr.shape[0]] = arr
        self.off[name] = (self.n, arr.shape[1])
        self.cols.append(a)
        self.n += arr.shape[1]

    def build(self):
        return np.concatenate(self.cols, axis=1)


def _host_pack(inp):
    """Returns (wp f32-staged-to-f32r, vp f32, bp bf16, flags)."""
    flags = {}
    dtb = np.asarray(inp["mp_dt_b"], np.float64)
    flags["dtb_const"] = float(dtb.flat[0]) if np.ptp(dtb) < 1e-12 else None
    flags["D_ones"] = bool(np.allclose(np.asarray(inp["mp_D"]), 1.0))
    flags["convb_zero"] = bool(np.all(np.asarray(inp["mp_conv_b"]) == 0.0))
    flags["f1b_zero"] = bool(np.all(np.asarray(inp["ff1_b"]) == 0.0))
    flags["f2b_zero"] = bool(np.all(np.asarray(inp["ff2_b"]) == 0.0))
    flags["an_id"] = bool(np.all(np.asarray(inp["an_g"]) == 1.0)
                          and np.all(np.asarray(inp["an_b"]) == 0.0))
    flags["fln_id"] = bool(np.all(np.asarray(inp["fln_g"]) == 1.0)
                           and np.all(np.asarray(inp["fln_b"]) == 0.0))
    flags["mixb_zero"] = bool(np.all(np.asarray(inp["mix_b"]) == 0.0))
    A = -np.exp(np.asarray(inp["mp_Alog"], np.float64))      # (3, DI, S)
    flags["A_shared"] = bool(
        np.ptp(A, axis=(0, 1)).max() < 1e-9 * np.abs(A).max())

    bp = Pack()   # bf16 weights
    for g in range(16):
        sm = np.zeros((128, 128), np.float32)
        for k in range(128):
            sm[k, g * 8 + k // 16] = 1.0
        bp.add(f"sum{g}", sm)

    def delta_A(Am, cc, g):
        # out partition j <- A[cc*128 + g*8 + j//16, j%16] placed at input
        # row g*8 + j//16 (row of the per-cc dts tile).
        dl = np.zeros((128, 128), np.float32)
        for j in range(128):
            dl[g * 8 + j // 16, j] = Am[cc * 128 + g * 8 + j // 16, j % 16]
        return dl

    if flags["A_shared"]:
        for g in range(16):
            bp.add(f"dA{g}", delta_A(A[0], 0, g))
    else:
        for m in range(3):
            for cc in range(2):
                for g in range(16):
                    bp.add(f"dA{m}{cc}{g}", delta_A(A[m], cc, g))

    wp = Pack()   # fp32 staged -> f32r on device
    vp = Pack()   # fp32 per-partition vectors
    for m in range(3):
        bp.add(f"win{m}", inp["mp_in"][m])                   # (D, 512)
        wxp = inp["mp_xproj"][m]                             # (DI, 40)
        for cc in range(2):
            bp.add(f"bc{m}{cc}", wxp[cc * 128:(cc + 1) * 128])   # (128, 40)
        dtw = inp["mp_dt_w"][m]                              # (R, DI)
        for cc in range(2):
            bp.add(f"dtw{m}{cc}", dtw[:, cc * 128:(cc + 1) * 128])
        wout = inp["mp_out"][m]                              # (DI, D)
        for cc in range(2):
            bp.add(f"wout{m}{cc}", wout[cc * 128:(cc + 1) * 128])
        bp.add(f"ff1{m}", inp["ff1_w"][m])                   # (D, 512)
        ff2 = inp["ff2_w"][m]                                # (4D, D)
        for c4 in range(4):
            bp.add(f"ff2{m}{c4}", ff2[c4 * 128:(c4 + 1) * 128])
    mixw = inp["mix_w"]
    for kc in range(2):
        for mc in range(2):
            bp.add(f"mix{kc}{mc}", mixw[kc * 128:(kc + 1) * 128,
                                        mc * 128:(mc + 1) * 128])
    wp.add("onesD", np.full((128, 1), 1.0 / D, np.float32))
    bp.add("onesDb", np.full((128, 1), 1.0 / D, np.float32))
    wp.add("ones1", np.ones((1, 128), np.float32))

    vp.add("eps", np.full((128, 1), LN_EPS, np.float32))
    if flags["dtb_const"] is not None:
        vp.add("dtbc", np.full((128, 1), flags["dtb_const"], np.float32))
    vp.add("I64", np.eye(64, dtype=np.float32))
    vp.add("I128", np.eye(128, dtype=np.float32))
    for m in range(3):
        cw = inp["mp_conv_w"][m]
        for cc in range(2):
            sl = slice(cc * 128, (cc + 1) * 128)
            vp.add(f"cw{m}{cc}", cw[sl])                     # 4 cols
            if not flags["convb_zero"]:
                vp.add(f"cb{m}{cc}", inp["mp_conv_b"][m][sl, None])
            if flags["dtb_const"] is None:
                vp.add(f"dtb{m}{cc}", inp["mp_dt_b"][m][sl, None])
            if not flags["D_ones"]:
                vp.add(f"Dp{m}{cc}", inp["mp_D"][m][sl, None])
        if not flags["f1b_zero"]:
            for c4 in range(4):
                vp.add(f"f1b{m}{c4}",
                       inp["ff1_b"][m][c4 * 128:(c4 + 1) * 128, None])
        if not flags["f2b_zero"]:
            vp.add(f"f2b{m}", inp["ff2_b"][m][:, None])
        if not flags["an_id"]:
            vp.add(f"ang{m}", inp["an_g"][m][:, None])
            vp.add(f"anb{m}", inp["an_b"][m][:, None])
        if not flags["fln_id"]:
            vp.add(f"flg{m}", inp["fln_g"][m][:, None])
            vp.add(f"flb{m}", inp["fln_b"][m][:, None])
    if not flags["mixb_zero"]:
        for mc in range(2):
            vp.add(f"mixb{mc}", inp["mix_b"][mc * 128:(mc + 1) * 128, None])
    return wp, vp, bp, flags


def _emit(ctx, tc, nc, aps, wp, vp, bpk, flags):
    wpool = ctx.enter_context(tc.tile_pool(name="weights", bufs=1))
    wr = wpool.tile([128, wp.n], f32r, name="wr", tag="wr")
    vec = wpool.tile([128, vp.n], f32, name="vec", tag="vec")
    nc.sync.dma_start(vec[:], aps["vpack"][:])
    bw = wpool.tile([128, bpk.n], bf16, name="bw", tag="bw")
    nc.sync.dma_start(bw[:], aps["bpack"][:])
    with tc.tile_pool(name="wstage", bufs=1) as stpool:
        wstage = stpool.tile([128, wp.n], f32, name="wstage")
        nc.sync.dma_start(wstage[:], aps["wpack"][:])
        for o in range(0, wp.n, 8192):
            e = min(wp.n, o + 8192)
            nc.vector.tensor_copy(wr[:, o:e], wstage[:, o:e])

    def WR(name):
        o, c = wp.off[name]
        return wr[:, o:o + c]

    def VP(name):
        o, c = vp.off[name]
        return vec[:, o:o + c]

    def BR(name):
        o, c = bpk.off[name]
        return bw[:, o:o + c]

    def mm(psum_ap, lhsT_ap, rhs_ap, start, stop, kp=128):
        nc.tensor.matmul(psum_ap, lhsT_ap[:kp, :], rhs_ap[:kp, :],
                         start=start, stop=stop)

    def dAW(mi, cc, g):
        return BR(f"dA{g}" if flags["A_shared"] else f"dA{mi}{cc}{g}")

    I64 = VP("I64")
    I128 = VP("I128")

    statA = {bname: aps[f"scr_stA_{bname}"] for bname, _, _, _, _ in BLOCKS}

    stpool = ctx.enter_context(tc.tile_pool(name="stat_s", bufs=2))

    def ln_stats(src_ap, sq_ap, statT, c0, ppool, ptag, pbufs=2):
        """src (128,CT); sq squared values; write stats into statT."""
        ob = BR("onesDb")[:, 0:1]
        of = WR("onesD")[:, 0:1]
        pmq = ppool.tile([1, 2 * CT], f32, name="pmq", tag=ptag, bufs=pbufs)
        mm(pmq[:, 0:CT], ob if src_ap.dtype == bf16 else of, src_ap,
           True, True)
        mm(pmq[:, CT:2 * CT], ob if sq_ap.dtype == bf16 else of, sq_ap,
           True, True)
        sst = stpool.tile([1, 2 * CT], f32, name="sst", tag="sst")
        nc.scalar.activation(sst[:], pmq[:], AF.Copy)
        p0 = c0 // NW
        nc.sync.dma_start(
            statT[p0:p0 + 8].rearrange("p h w -> h p w"),
            sst[:].rearrange("x (h p w) -> x h p w", h=2, p=8))

    # ================= phase A =====================================
    def phase_a(bname, mi, xkey, q_off, sqp):
        is_t = bname in ("t0", "t1")
        xcb = [sqp.tile([128, CBLK], bf16, name=f"xcb{cc}", tag=f"xcb{cc}")
               for cc in range(2)]
        zsb = [sqp.tile([128, CBLK], bf16, name=f"zsb{cc}", tag=f"zsb{cc}")
               for cc in range(2)]
        xT = sqp.tile([128, CBLK], bf16, name="xT", tag="xT")

        # ---- A1 (with fused per-tile input transpose) ------------
        with tc.tile_pool(name="a0", bufs=3) as a0p, \
             tc.tile_pool(name="a0p", bufs=2, space="PSUM") as a0pp, \
             tc.tile_pool(name="a1", bufs=2) as a1p, \
             tc.tile_pool(name="a1pxc", bufs=1, space="PSUM") as pxc, \
             tc.tile_pool(name="a1pz", bufs=1, space="PSUM") as pz_, \
             tc.tile_pool(name="a1pbc", bufs=2, space="PSUM") as pbcp:
            for c0 in range(0, CBLK, CT):
                q0 = c0 // W
                raw = a0p.tile([64, 8 * 128], f32, name="raw", tag="raw")
                nc.sync.dma_start(raw[:],
                                  aps[xkey][:, q_off + q0:q_off + q0 + 8, :])
                pt = a0pp.tile([128, 8 * 64], f32, name="pt", tag="pt")
                for i in range(8):
                    nc.tensor.transpose(pt[:, i * 64:(i + 1) * 64],
                                        raw[:, i * 128:(i + 1) * 128],
                                        I64[:64, :64])
                nc.scalar.activation(xT[:, c0:c0 + CT], pt[:], AF.Copy)
                xt_t = xT[:, c0:c0 + CT]
                pxc2 = pxc.tile([128, 2 * CT], f32, name="pxc2", tag="pxc2")
                for cc in range(2):
                    mm(pxc2[:, cc * CT:(cc + 1) * CT],
                       BR(f"win{mi}")[:, cc * 128:(cc + 1) * 128], xt_t,
                       True, True)
                pz2 = pz_.tile([128, 2 * CT], f32, name="pz2", tag="pz2")
                for cc in range(2):
                    mm(pz2[:, cc * CT:(cc + 1) * CT],
                       BR(f"win{mi}")[:, (2 + cc) * 128:(3 + cc) * 128],
                       xt_t, True, True)
                for cc in range(2):
                    nc.scalar.activation(zsb[cc][:, c0:c0 + CT],
                                         pz2[:, cc * CT:(cc + 1) * CT],
                                         AF.Silu)
                zc = a1p.tile([128, 2 * CT], bf16, name="zc", tag="zc")
                nc.scalar.activation(zc[:], pxc2[:], AF.Copy)
                acc2 = a1p.tile([128, 2 * CT], f32, name="acc2", tag="acc2")
                for cc in range(2):
                    pzv = zc[:, cc * CT:(cc + 1) * CT]
                    accv = acc2[:, cc * CT:(cc + 1) * CT]
                    cw = VP(f"cw{mi}{cc}")
                    srcr = pzv.rearrange("p (q t) -> p q t", t=W)
                    accr = accv.rearrange("p (q t) -> p q t", t=W)
                    nc.vector.tensor_scalar(accv, pzv, cw[:, 3:4], None,
                                            OP.mult)
                    for k in range(3):
                        sh = 3 - k
                        eng = nc.vector
                        eng.scalar_tensor_tensor(
                            accr[:, :, sh:W], srcr[:, :, 0:W - sh],
                            cw[:, k:k + 1], accr[:, :, sh:W],
                            OP.mult, OP.add)
                for cc in range(2):
                    bias = (None if flags["convb_zero"]
                            else VP(f"cb{mi}{cc}"))
                    if bias is None:
                        nc.scalar.activation(xcb[cc][:, c0:c0 + CT],
                                             acc2[:, cc * CT:(cc + 1) * CT],
                                             AF.Silu)
                    else:
                        nc.scalar.activation(xcb[cc][:, c0:c0 + CT],
                                             acc2[:, cc * CT:(cc + 1) * CT],
                                             AF.Silu, bias=bias)
                pbc = pbcp.tile([40, CT], f32, name="pbc", tag="pbc")
                for cc in range(2):
                    mm(pbc[:], BR(f"bc{mi}{cc}"),
                       xcb[cc][:, c0:c0 + CT], cc == 0, cc == 1)
                bcs = a1p.tile([40, CT], bf16, name="bcs", tag="bcs")
                nc.scalar.activation(bcs[:], pbc[:], AF.Copy)
                nc.sync.dma_start(aps[f"scr_bc_{bname}"][:, c0:c0 + CT],
                                  bcs[:])

        # ---- A2 loop 1: dt factory (Exp/Ln batched by 4) ---------
        with tc.tile_pool(name="f1", bufs=2) as f1p, \
             tc.tile_pool(name="f1ps", bufs=2, space="PSUM") as f1ps:
            for c00 in range(0, CBLK, 4 * CT):
                ezs = []
                for ci in range(4):
                    c0 = c00 + ci * CT
                    dtin = f1p.tile([8, CT], bf16, name="dtin", tag="dtin",
                                    bufs=4)
                    nc.sync.dma_start(dtin[:],
                                      aps[f"scr_bc_{bname}"][0:8, c0:c0 + CT])
                    pd = f1ps.tile([128, 2 * CT], f32, name="pd", tag="pd",
                                   bufs=2)
                    for cc in range(2):
                        mm(pd[:, cc * CT:(cc + 1) * CT], BR(f"dtw{mi}{cc}"),
                           dtin[:], True, True, kp=8)
                    ez = f1p.tile([128, 2 * CT], bf16, name="ez", tag="ez",
                                  bufs=4)
                    if flags["dtb_const"] is not None:
                        nc.scalar.activation(ez[:], pd[:], AF.Exp,
                                             bias=VP("dtbc"))
                    else:
                        for cc in range(2):
                            nc.scalar.activation(
                                ez[:, cc * CT:(cc + 1) * CT],
                                pd[:, cc * CT:(cc + 1) * CT],
                                AF.Exp, bias=VP(f"dtb{mi}{cc}"))
                    ezs.append(ez)
                dtss = []
                for ci in range(4):
                    dts = f1p.tile([128, 2 * CT], bf16, name="dts",
                                   tag="dts", bufs=4)
                    nc.scalar.activation(dts[:], ezs[ci][:], AF.Ln, bias=1.0)
                    dtss.append(dts)
                for ci in range(4):
                    c0 = c00 + ci * CT
                    dts = dtss[ci]
                    dtx = f1p.tile([128, 2 * CT], bf16, name="dtx",
                                   tag="dtx", bufs=4)
                    for cc in range(2):
                        nc.vector.tensor_mul(dtx[:, cc * CT:(cc + 1) * CT],
                                             dts[:, cc * CT:(cc + 1) * CT],
                                             xcb[cc][:, c0:c0 + CT])
                    for cc in range(2):
                        nc.sync.dma_start(
                            aps[f"scr_dtx_{bname}"][cc, c0 // CT]
                            .rearrange("p (g c) -> g p c", g=16),
                            dtx[:, cc * CT:(cc + 1) * CT])
                    dtv = dts[:].rearrange("p (x t) -> p x t", t=W)
                    nc.vector.tensor_scalar(dtv[:, :, 0:1], dtv[:, :, 0:1],
                                            0.0, 1.0e4, OP.mult, OP.add)
                    nc.sync.dma_start(aps[f"scr_dts_{bname}"][:, :, c0:c0 + CT],
                                      dts[:].rearrange("p (cc c) -> p cc c",
                                                       cc=2))

        # ---- A2 loop 2: selective scan (exp table only) ----------
        with tc.tile_pool(name="a2", bufs=2) as a2p, \
             tc.tile_pool(name="a2u", bufs=4) as a2u, \
             tc.tile_pool(name="a2x", bufs=6) as a2x, \
             tc.tile_pool(name="a2pda", bufs=2, space="PSUM") as pda, \
             tc.tile_pool(name="a2ppy", bufs=2, space="PSUM") as ppy, \
             tc.tile_pool(name="a2pmisc", bufs=1, space="PSUM") as pmsc:
            for c0 in range(0, CBLK, CT):
                dtl = a2p.tile([128, 2 * CT], bf16, name="dtl", tag="dtl",
                               bufs=4)
                nc.sync.dma_start(
                    dtl[:].rearrange("p (cc c) -> p cc c", cc=2),
                    aps[f"scr_dts_{bname}"][:, :, c0:c0 + CT])
                Brep = a2p.tile([128, CT], bf16, name="Brep", tag="Brep",
                                bufs=4)
                nc.sync.dma_start(
                    Brep[:],
                    aps[f"scr_bc_{bname}"][8:24, c0:c0 + CT]
                    .unsqueeze(0).broadcast_to((8, 16, CT)))
                Crep = a2p.tile([128, CT], bf16, name="Crep", tag="Crep",
                                bufs=4)
                nc.sync.dma_start(
                    Crep[:],
                    aps[f"scr_bc_{bname}"][24:40, c0:c0 + CT]
                    .unsqueeze(0).broadcast_to((8, 16, CT)))
                ues = []
                for cc in range(2):
                    halves = []
                    for hf in range(2):
                        ueh = a2x.tile([128, 8 * CT], bf16, name="ueh",
                                       tag="ueh", bufs=4)
                        nc.sync.dma_start(
                            ueh[:],
                            aps[f"scr_dtx_{bname}"]
                            [cc, c0 // CT, :, hf * 8 * CT:(hf + 1) * 8 * CT]
                            .unsqueeze(1).broadcast_to((8, 16, 8 * CT)))
                        halves.append(ueh)
                    ues.append(halves)
                Brep_b = Brep[:].unsqueeze(1).broadcast_to((128, 2, CT))
                Crep_b = Crep[:].unsqueeze(1).broadcast_to((128, 2, CT))
                gzs = []
                for cc in range(2):
                    pY = ppy.tile([128, CT], f32, name="pY", tag="pY",
                                  bufs=2)
                    if flags["D_ones"]:
                        # pre-load the D-skip term; the sum matmuls
                        # accumulate on top (start=False), removing the
                        # DVE add from the scan window.
                        nc.scalar.activation(pY[:],
                                             xcb[cc][:, c0:c0 + CT],
                                             AF.Copy)
                    stage = []
                    for gp in range(11):
                        if gp < 8:
                            pP = pda.tile([128, 2 * CT], f32, name="pP",
                                          tag="pP", bufs=2)
                            for i in range(2):
                                g = gp * 2 + i
                                mm(pP[:, i * CT:(i + 1) * CT],
                                   dAW(mi, cc, g),
                                   dtl[:, cc * CT:(cc + 1) * CT],
                                   True, True)
                            dA = a2u.tile([128, 2 * CT], bf16, name="dA",
                                          tag="dA", bufs=4)
                            nc.scalar.activation(dA[:], pP[:], AF.Exp)
                            u2 = a2u.tile([128, 2 * CT], bf16, name="u2",
                                          tag="u2", bufs=5)
                            ueh = ues[cc][gp // 4]
                            sl = (gp % 4) * 2 * CT
                            if gp % 4 == 3:
                                for i in range(2):
                                    nc.gpsimd.tensor_mul(
                                        u2[:, i * CT:(i + 1) * CT],
                                        ueh[:, sl + i * CT:sl + (i + 1) * CT],
                                        Brep[:])
                            else:
                                nc.vector.tensor_mul(
                                    u2[:].rearrange("p (i c) -> p i c", i=2),
                                    ueh[:, sl:sl + 2 * CT]
                                    .rearrange("p (i c) -> p i c", i=2),
                                    Brep_b)
                            stage.append((gp, dA, u2))
                        if gp >= 3:
                            gq, dAq, u2q = stage.pop(0)
                            h2 = a2u.tile([128, 2 * CT], bf16, name="h2",
                                          tag="h2", bufs=4)
                            for i in range(2):
                                nc.vector.tensor_tensor_scan(
                                    h2[:, i * CT:(i + 1) * CT],
                                    dAq[:, i * CT:(i + 1) * CT],
                                    u2q[:, i * CT:(i + 1) * CT],
                                    0.0, OP.mult, OP.add)
                            yh2 = a2u.tile([128, 2 * CT], bf16, name="yh2",
                                           tag="yh2", bufs=4)
                            if gq % 2 == 1:
                                for i in range(2):
                                    nc.gpsimd.tensor_mul(
                                        yh2[:, i * CT:(i + 1) * CT],
                                        h2[:, i * CT:(i + 1) * CT],
                                        Crep[:])
                            else:
                                nc.vector.tensor_mul(
                                    yh2[:].rearrange("p (i c) -> p i c",
                                                     i=2),
                                    h2[:].rearrange("p (i c) -> p i c",
                                                    i=2),
                                    Crep_b)
                            for i in range(2):
                                g = gq * 2 + i
                                mm(pY[:], BR(f"sum{g}"),
                                   yh2[:, i * CT:(i + 1) * CT],
                                   g == 0 and not flags["D_ones"],
                                   g == 15)
                    yg = a2p.tile([128, CT], bf16, name=f"yg{cc}",
                                  tag=f"yg{cc}")
                    if flags["D_ones"]:
                        # pY already holds sum+skip; drain on Act (slack)
                        nc.scalar.activation(yg[:], pY[:], AF.Copy)
                    else:
                        nc.vector.scalar_tensor_tensor(
                            yg[:], xcb[cc][:, c0:c0 + CT],
                            VP(f"Dp{mi}{cc}"), pY[:], OP.mult, OP.add)
                    gz = a2p.tile([128, CT], bf16, name=f"gz{cc}",
                                  tag=f"gz{cc}")
                    nc.gpsimd.tensor_mul(gz[:], yg[:],
                                         zsb[cc][:, c0:c0 + CT])
                    gzs.append(gz)
                po = pmsc.tile([128, CT], f32, name="po", tag="pdpo",
                               bufs=2)
                for cc in range(2):
                    mm(po[:], BR(f"wout{mi}{cc}"), gzs[cc][:],
                       cc == 0, cc == 1)
                res = a2p.tile([128, CT], bf16, name="res", tag="res")
                if is_t:
                    nc.vector.tensor_add(res[:], po[:], xT[:, c0:c0 + CT])
                    nc.sync.dma_start(aps[f"scr_s_{bname}"][:, c0:c0 + CT],
                                      res[:])
                    s2 = a2p.tile([128, CT], bf16, name="s2", tag="s2")
                    nc.scalar.activation(s2[:], res[:], AF.Square)
                    ln_stats(res[:], s2[:], statA[bname], c0, pda, "pP")
                else:
                    nc.scalar.activation(res[:], po[:], AF.Copy)
                    nc.sync.dma_start(aps[f"scr_ym_{bname}"][:, c0:c0 + CT],
                                      res[:])

    ABLOCKS = {b[0]: b for b in BLOCKS}
    with tc.tile_pool(name="a_sq", bufs=1) as a_sqp:
        for bname in ("n", "l"):
            bn, mi, xkey, q_off, _ = ABLOCKS[bname]
            phase_a(bn, mi, xkey, q_off, a_sqp)
        # ---- mix phase (n, l) -----------------------------------
        with tc.tile_pool(name="mixw", bufs=2) as mxp, \
             tc.tile_pool(name="mixp", bufs=1, space="PSUM") as mxpp:
            for c0 in range(0, CBLK, CT):
                q0 = c0 // W
                cat = []
                for bname in ("n", "l"):
                    t_ = mxp.tile([128, CT], bf16, name=f"ym{bname}",
                                  tag=f"ym{bname}")
                    nc.sync.dma_start(t_[:],
                                      aps[f"scr_ym_{bname}"][:, c0:c0 + CT])
                    cat.append(t_)
                for mc, bname in enumerate(("n", "l")):
                    pmx = mxpp.tile([128, CT], f32, name="pmx", tag="pmx",
                                    bufs=2)
                    for kc in range(2):
                        mm(pmx[:], BR(f"mix{kc}{mc}"), cat[kc][:], kc == 0,
                           kc == 1)
                    ms = mxp.tile([128, CT], bf16, name="ms", tag="ms")
                    if flags["mixb_zero"]:
                        nc.scalar.activation(ms[:], pmx[:], AF.Silu)
                    else:
                        nc.scalar.activation(ms[:], pmx[:], AF.Silu,
                                             bias=VP(f"mixb{mc}"))
                    raw = mxp.tile([64, 8 * 128], f32, name="rawm",
                                   tag="rawm")
                    nc.sync.dma_start(raw[:],
                                      aps[f"x_{bname}"][:, q0:q0 + 8, :])
                    ptx = mxpp.tile([128, CT], f32, name="ptx", tag="pmx",
                                    bufs=2)
                    for i in range(8):
                        nc.tensor.transpose(ptx[:, i * 64:(i + 1) * 64],
                                            raw[:, i * 128:(i + 1) * 128],
                                            I64[:64, :64])
                    t2 = mxp.tile([128, CT], bf16, name="t2", tag="t2")
                    nc.vector.tensor_add(t2[:], cat[mc][:], ms[:])
                    res = mxp.tile([128, CT], bf16, name="resm", tag="resm")
                    nc.vector.tensor_add(res[:], t2[:], ptx[:])
                    nc.sync.dma_start(aps[f"scr_s_{bname}"][:, c0:c0 + CT],
                                      res[:])
                    s2m = mxp.tile([128, CT], bf16, name="s2m", tag="s2m")
                    nc.scalar.activation(s2m[:], res[:], AF.Square)
                    ln_stats(res[:], s2m[:], statA[bname], c0, mxpp, "pmx")
        for bname in ("t0", "t1"):
            bn, mi, xkey, q_off, _ = ABLOCKS[bname]
            phase_a(bn, mi, xkey, q_off, a_sqp)

    # ================= phase C =====================================
    def ln_finish(tag, statT, pool, scr):
        stl = pool.tile([128, 2 * NW], f32, name=f"stl_{tag}", tag="stl")
        nc.sync.dma_start(stl[:].rearrange("p (h w) -> p h w", h=2),
                          statT[:])
        m_t, q_t = stl[:, 0:NW], stl[:, NW:2 * NW]
        var = pool.tile([128, NW], f32, name=f"var_{tag}", tag="lnvar")
        nc.vector.tensor_mul(var[:], m_t, m_t)
        nc.vector.tensor_sub(var[:], q_t, var[:])
        sd = pool.tile([128, NW], f32, name=f"sd_{tag}", tag="lnsd")
        nc.scalar.activation(sd[:], var[:], AF.Sqrt, bias=VP("eps"))
        r_t = pool.tile([128, NW], f32r, name=f"r_{tag}", tag="lnr")
        with nc.allow_low_precision(reason="f32r LN scale factors"):
            nc.vector.reciprocal(r_t[:], sd[:])
        nmr = pool.tile([128, NW], f32r, name=f"nmr_{tag}", tag="lnnmr")
        nc.vector.tensor_mul(nmr[:], m_t, r_t[:])
        nc.vector.tensor_scalar(nmr[:], nmr[:], -1.0, None, OP.mult)
        rnm = pool.tile([128, 2 * NW], bf16, name=f"rnm_{tag}", tag="lnrnm")
        nc.vector.tensor_copy(rnm[:, 0:NW], r_t[:])
        nc.vector.tensor_copy(rnm[:, NW:2 * NW], nmr[:])
        nc.sync.dma_start(scr.rearrange("h p w -> p h w"), rnm[:])
        return r_t, nmr

    def ln_apply(src_ap, scr, c0, gk, bk, out_ap, pool):
        p0 = c0 // NW
        rnm2 = pool.tile([128, 2 * CT], bf16, name="rnm2", tag="rnm2",
                         bufs=3)
        nc.sync.dma_start(
            rnm2[:].rearrange("j (h ab) -> j h ab", h=2),
            scr[:, p0:p0 + 8, :].rearrange("h p w -> h (p w)")
            .unsqueeze(0).broadcast_to((128, 2, CT)))
        t1 = pool.tile([128, CT], bf16, name="t1", tag="t1")
        nc.vector.tensor_mul(t1[:], src_ap, rnm2[:, 0:CT])
        if gk is None:
            nc.vector.tensor_add(out_ap, t1[:], rnm2[:, CT:2 * CT])
        else:
            nc.vector.tensor_add(t1[:], t1[:], rnm2[:, CT:2 * CT])
            nc.vector.tensor_scalar(out_ap, t1[:], VP(gk), VP(bk),
                                    OP.mult, OP.add)

    with tc.tile_pool(name="c_per", bufs=1) as cper, \
         tc.tile_pool(name="c_w", bufs=2) as cp, \
         tc.tile_pool(name="c_p", bufs=1, space="PSUM") as cpp, \
         tc.tile_pool(name="c_pf", bufs=1, space="PSUM") as cpf:
        state = {}

        def c_front(bname, mi):
            ln_finish(f"a{bname}", statA[bname], cp,
                      aps[f"scr_ln_{bname}_a"][:])
            statF = aps[f"scr_stF_{bname}"]
            n1 = cper.tile([128, CBLK], bf16, name=f"n1_{bname}",
                           tag="napply", bufs=2)
            sf = cper.tile([128, CBLK], bf16, name=f"sf_{bname}",
                           tag="sfb", bufs=3)
            for c0 in range(0, CBLK, CT):
                sld = cp.tile([128, CT], bf16, name="sld", tag="sld",
                              bufs=3)
                nc.sync.dma_start(sld[:],
                                  aps[f"scr_s_{bname}"][:, c0:c0 + CT])
                ga, gb = (None, None) if flags["an_id"] else \
                    (f"ang{mi}", f"anb{mi}")
                ln_apply(sld[:], aps[f"scr_ln_{bname}_a"], c0, ga, gb,
                         n1[:, c0:c0 + CT], cp)
                hh = cp.tile([128, 4 * CT], bf16, name="hh", tag="hh")
                for hp in range(2):
                    pf = cpf.tile([128, 2 * CT], f32, name="pf", tag="pf",
                                  bufs=2)
                    for ci in range(2):
                        c4 = hp * 2 + ci
                        mm(pf[:, ci * CT:(ci + 1) * CT],
                           BR(f"ff1{mi}")[:, c4 * 128:(c4 + 1) * 128],
                           n1[:, c0:c0 + CT], True, True)
                    if flags["f1b_zero"]:
                        nc.scalar.activation(
                            hh[:, hp * 2 * CT:(hp + 1) * 2 * CT], pf[:],
                            AF.Prelu, alpha=0.01)
                    else:
                        for ci in range(2):
                            c4 = hp * 2 + ci
                            nc.scalar.activation(
                                hh[:, c4 * CT:(c4 + 1) * CT],
                                pf[:, ci * CT:(ci + 1) * CT], AF.Prelu,
                                bias=VP(f"f1b{mi}{c4}"), alpha=0.01)
                pf2 = cpp.tile([128, CT], f32, name="pf2", tag="pf2",
                               bufs=2)
                for c4 in range(4):
                    mm(pf2[:], BR(f"ff2{mi}{c4}"),
                       hh[:, c4 * CT:(c4 + 1) * CT], c4 == 0, c4 == 3)
                if flags["f2b_zero"]:
                    nc.vector.tensor_add(sf[:, c0:c0 + CT], pf2[:],
                                         n1[:, c0:c0 + CT])
                else:
                    nc.vector.scalar_tensor_tensor(sf[:, c0:c0 + CT],
                                                   pf2[:], VP(f"f2b{mi}"),
                                                   n1[:, c0:c0 + CT],
                                                   OP.add, OP.add)
                s2f = cp.tile([128, CT], bf16, name="s2f", tag="s2f")
                nc.scalar.activation(s2f[:], sf[:, c0:c0 + CT],
                                     AF.Square)
                ln_stats(sf[:, c0:c0 + CT], s2f[:], statF, c0, cpf, "pf",
                         pbufs=2)
            state[bname] = (statF, sf)

        def c_tail(bname, mi, j0):
            statF, sf = state.pop(bname)
            ln_finish(f"f{bname}", statF, cp,
                      aps[f"scr_ln_{bname}_f"][:])
            n2 = cper.tile([128, CBLK], f32, name=f"n2_{bname}",
                           tag="napply", bufs=2)
            for c0 in range(0, CBLK, CT):
                ga, gb = (None, None) if flags["fln_id"] else \
                    (f"flg{mi}", f"flb{mi}")
                ln_apply(sf[:, c0:c0 + CT], aps[f"scr_ln_{bname}_f"], c0,
                         ga, gb, n2[:, c0:c0 + CT], cp)
            for q0 in range(0, Q, 8):
                c0 = q0 * 64
                pt = cpp.tile([128, 512], f32, name="pto", tag="pto",
                              bufs=2)
                for c in range(4):
                    nc.tensor.transpose(
                        pt[:, c * 128:(c + 1) * 128],
                        n2[:, c0 + c * 128:c0 + (c + 1) * 128], I128)
                ot = cp.tile([128, 512], f32, name="ot", tag="ot")
                nc.vector.tensor_copy(ot[:], pt[:])
                for qh in range(2):
                    nc.sync.dma_start(
                        aps["out"][:, j0 + q0:j0 + q0 + 8, :]
                        .rearrange("t (c q) d -> q t c d", c=4)[qh],
                        ot[qh * 64:(qh + 1) * 64, :]
                        .rearrange("t (c d) -> t c d", c=4))

        binfo = {b[0]: b for b in BLOCKS}
        order = ["n", "t0", "t1", "l"]
        prev = None
        for bname in order:
            _, mi, _, _, j0 = binfo[bname]
            c_front(bname, mi)
            if prev is not None:
                _, pmi, _, _, pj0 = binfo[prev]
                c_tail(prev, pmi, pj0)
            prev = bname
        _, pmi, _, _, pj0 = binfo[prev]
        c_tail(prev, pmi, pj0)


def _build_program(wp, vp, bpk, flags):
    nc = bacc.Bacc("TRN2", target_bir_lowering=False, debug=False,
                   num_devices=N_CORES)
    aps = {}
    aps["x_n"] = nc.dram_tensor("x_n", [W, N, D], f32,
                                kind="ExternalInput").ap()
    aps["x_t"] = nc.dram_tensor("x_t", [W, E, D], f32,
                                kind="ExternalInput").ap()
    aps["x_l"] = nc.dram_tensor("x_l", [W, N, D], f32,
                                kind="ExternalInput").ap()
    aps["wpack"] = nc.dram_tensor("wpack", [128, wp.n], f32,
                                  kind="ExternalInput").ap()
    aps["vpack"] = nc.dram_tensor("vpack", [128, vp.n], f32,
                                  kind="ExternalInput").ap()
    aps["bpack"] = nc.dram_tensor("bpack", [128, bpk.n], bf16,
                                  kind="ExternalInput").ap()
    aps["out"] = nc.dram_tensor("out", [W, 2 * N + E, D], f32,
                                kind="ExternalOutput").ap()
    for bname, _, _, _, _ in BLOCKS:
        aps[f"scr_bc_{bname}"] = nc.dram_tensor(
            f"scr_bc_{bname}", [40, CBLK], bf16).ap()
        aps[f"scr_dtx_{bname}"] = nc.dram_tensor(
            f"scr_dtx_{bname}", [2, CBLK // CT, 8, 16 * CT], bf16).ap()
        aps[f"scr_stA_{bname}"] = nc.dram_tensor(
            f"scr_stA_{bname}", [128, 2, NW], f32).ap()
        aps[f"scr_stF_{bname}"] = nc.dram_tensor(
            f"scr_stF_{bname}", [128, 2, NW], f32).ap()
        aps[f"scr_ln_{bname}_a"] = nc.dram_tensor(
            f"scr_ln_{bname}_a", [2, 128, NW], bf16).ap()
        aps[f"scr_ln_{bname}_f"] = nc.dram_tensor(
            f"scr_ln_{bname}_f", [2, 128, NW], bf16).ap()
        aps[f"scr_dts_{bname}"] = nc.dram_tensor(
            f"scr_dts_{bname}", [128, 2, CBLK], bf16).ap()
        aps[f"scr_s_{bname}"] = nc.dram_tensor(
            f"scr_s_{bname}", [128, CBLK], bf16).ap()
        if bname in ("n", "l"):
            aps[f"scr_ym_{bname}"] = nc.dram_tensor(
                f"scr_ym_{bname}", [128, CBLK], bf16).ap()

    with tile.TileContext(nc) as tc:
        with ExitStack() as ctx:
            _emit(ctx, tc, nc, aps, wp, vp, bpk, flags)
    nc.compile()
    return nc


_CACHE = {}


def kernel(**inputs):
    wp, vp, bpk, flags = _host_pack(inputs)
    if "prog" not in _CACHE:
        _CACHE["prog"] = _build_program(wp, vp, bpk, flags)
    nc = _CACHE["prog"]
    wpack, vpack = wp.build(), vp.build()
    bpack = bpk.build().astype(ml_dtypes.bfloat16)
    in_maps = []
    for b in range(B):
        in_maps.append({
            "x_n": np.ascontiguousarray(inputs["x_node"][b]),
            "x_t": np.ascontiguousarray(inputs["x_trace"][b]),
            "x_l": np.ascontiguousarray(inputs["x_log"][b]),
            "wpack": wpack,
            "vpack": vpack,
            "bpack": bpack,
        })
    res = run_bass_kernel_spmd(nc, in_maps, list(range(N_CORES)))
    out = np.stack([res.results[b]["out"] for b in range(B)], axis=0)
    return out.astype(np.float32)

